# revision 24
# baseline (speedup 1.0000x reference)
"""PointerNet attention scoring kernel for Trainium2 (8 NeuronCores).

Computes, for full inputs:
    q_t = query @ Wq + bq                      # (L_q, B, H)
    h_t = decoder_states @ Wh + bh             # (L_a, B, H)
    s[a,q,b] = sum_h tanh(q_t[q,b,h] + h_t[a,b,h]) * w2[h] (+ b2)
    out[a,b,q] = softmax_q(s[a,q,b])  (mask applied post-exp; ones here)

Sharding: data-parallel over L_a (512 -> 8 x 64). Each core receives the
full (host-pre-arranged, partition-major) query / weights and its
decoder_states slice, and produces a row-permuted (256, 512) block that
the host scatters into the (64, B, L_q) output slice. b2 is dropped
(softmax-invariant); the query mask, if not all ones, is applied
host-side (exactly). Host prep is layout-only - all FLOPs stay on
device.

Per-core on-chip pipeline (raw Bass, explicit semaphores - the walrus
build here only accepts one embedded sync-wait per instruction, so Tile
is unusable and all cross-engine waits are standalone wait_ge):
  - H=128 on partitions. q_tT[h,q] per b and bias columns
    h_tT[h,(b,a)]+bq+bh from small fp32 PE matmuls; stored bf16/f32.
  - Main loop, 8 chunks of CH=32 (a,b) pairs (first/last chunk split in
    two for pipeline ramp), bf16 datapath: DVE tensor_scalar_add
    broadcasts a bias column over q; one in-place ScalarE Tanh per
    chunk-part (ScalarE is the roofline: 16.8M elems / 128 lanes /
    1.2 GHz ~= 109 us); PE reduces each pair with a one-hot-scaled bf16
    w2 stationary ([128,32], w2 in column v) at tile_position (0,32j),
    accumulating into PSUM partition 32j+v of per-column-group banks
    (the 31 zero stationary columns add exact +0.0; bf16 matvecs are
    single-pass where fp32 would be two).
  - Scores tile 0 interleaves its matvecs over all 4 column-groups
    (4-way PE concurrency); its softmax hides under the next tanh.
    Tile 1 fills groups {0,1} during chunks 4-5 and {2,3} during 6-7
    (2-way concurrency) so half its softmax also hides under tanh and
    only groups 2,3 drain at the kernel tail.
  - Softmax over q (free axis), fp32, per 32-row bank piece: DVE
    negated max, ScalarE Exp with bias=-max and fused row-sum accum,
    DVE reciprocal + scale, 128/256 KB output DMAs.
"""

import numpy as np

L_Q, L_A, B = 512, 512, 4
Q_SIZE, D_SIZE, H = 256, 512, 128
N_CORES = 8
A_PER = L_A // N_CORES  # 64
CH = 32                 # (a,b) pairs per tanh chunk
NCHUNK = (A_PER * B) // CH          # 8
NTILE = (A_PER * B) // 128          # 2 scores tiles of 128 pair-rows
NAB = A_PER * B                     # 256 pair rows
GPT = 128 // CH                     # 4 chunks per scores tile
CPB = A_PER // CH                   # 2 chunks per batch entry
NQC = Q_SIZE // 128                 # 2 contraction chunks for q_t
NDC = D_SIZE // 128                 # 4 contraction chunks for h_t
NWC = NQC + NDC

_CACHE = {}


def _parts_of(g):
    return 4 if g in (0, NCHUNK - 1) else 1


def _part_ks(g, pt):
    n = _parts_of(g)
    lo = pt * (CH // n)
    return range(lo, lo + CH // n)


def _mm_plan(g, k):
    """(bank j, one-hot column v, start, stop) for pair-block k of chunk g."""
    t, gt = divmod(g, GPT)
    if t == 0:
        j = k % 4
        v = 8 * gt + k // 4
        return j, v, (gt == 0 and k < 4), (gt == GPT - 1 and k >= CH - 4)
    j = 2 * (gt // 2) + k % 2
    v = 16 * (gt % 2) + k // 2
    return j, v, (gt % 2 == 0 and k < 2), (gt % 2 == 1 and k >= CH - 2)


def _row_perm():
    """perm[a, b] = raw row index holding out[a, b, :]."""
    perm = np.empty((A_PER, B), dtype=np.int64)
    for g in range(NCHUNK):
        t = g // GPT
        b = g // CPB
        for k in range(CH):
            a = (g % CPB) * CH + k
            j, v, _, _ = _mm_plan(g, k)
            perm[a, b] = t * 128 + 32 * j + v
    return perm


def build_program():
    from contextlib import ExitStack

    import concourse.bass as bass
    from concourse import mybir

    f32 = mybir.dt.float32
    bf16 = mybir.dt.bfloat16
    AF = mybir.ActivationFunctionType
    ALU = mybir.AluOpType
    AX = mybir.AxisListType

    nc = bass.Bass()
    qT = nc.declare_dram_parameter("qT", [B, 128, NQC, L_Q], f32, isOutput=False)
    dT = nc.declare_dram_parameter("dT", [128, NDC, NAB], f32, isOutput=False)
    wqh = nc.declare_dram_parameter("wqh", [128, NWC, H], f32, isOutput=False)
    w2oh_in = nc.declare_dram_parameter("w2oh", [H, 32, 32], bf16, isOutput=False)
    bqh = nc.declare_dram_parameter("bqh", [H, 1], f32, isOutput=False)
    raw = nc.declare_dram_parameter("raw", [NAB, L_Q], f32, isOutput=True)

    with ExitStack() as ctx:
        _n = [0]

        def sb(shape, dt=f32):
            _n[0] += 1
            return ctx.enter_context(nc.sbuf_tensor(f"sb{_n[0]}", shape, dt))

        def ps(shape):
            _n[0] += 1
            return ctx.enter_context(nc.psum_tensor(f"ps{_n[0]}", shape, f32))

        wqh_sb = sb([128, NWC, H])
        w2oh = sb([128, 32, 32], bf16)
        bqh_sb = sb([128, 1])
        qT_sb = [sb([128, NQC, L_Q]) for _ in range(B)]
        dT_sb = sb([128, NDC, NAB])
        qtt = [sb([128, L_Q], bf16) for _ in range(B)]
        biasc = sb([128, NAB])  # fp32: tensor_scalar scalar1 must be f32
        tin = [sb([128, CH * L_Q], bf16) for _ in range(3)]
        probs = [sb([128, L_Q]) for _ in range(NTILE)]
        outt = [sb([128, L_Q]) for _ in range(NTILE)]
        sc0 = sb([128, L_Q])  # tile-0 scores gathered from the 4 banks
        negmax = [sb([128, 1]) for _ in range(NTILE)]
        sumexp = [sb([128, 1]) for _ in range(NTILE)]
        rsum = [sb([128, 1]) for _ in range(NTILE)]

        qt_ps = [ps([128, L_Q]) for _ in range(2)]
        ht_ps = ps([128, L_Q])   # prep uses [:, :NAB]; later a score bank
        spare_ps = ps([128, L_Q])
        banks = [ps([128, L_Q]) for _ in range(4)]  # tile-0 col-group scores
        banks1 = [qt_ps[0], qt_ps[1], ht_ps, spare_ps]  # tile-1 reuses prep

        wsem = ctx.enter_context(nc.semaphore("wsem"))
        qsem = [ctx.enter_context(nc.semaphore(f"qsem{b}")) for b in range(B)]
        dtsem = ctx.enter_context(nc.semaphore("dtsem"))
        bqsem = ctx.enter_context(nc.semaphore("bqsem"))
        w2sem = ctx.enter_context(nc.semaphore("w2sem"))
        psem = ctx.enter_context(nc.semaphore("psem"))
        asem = ctx.enter_context(nc.semaphore("asem"))
        vsem = ctx.enter_context(nc.semaphore("vsem"))
        osem = ctx.enter_context(nc.semaphore("osem"))

        # --- semaphore milestones (mirror each engine's program order)
        # psem: qt b0 (1), ht (2), qt b1..b3 (3..5), then per chunk-part
        pc = 0
        pc += 1
        p_ht = pc
        p_qt = {}
        for b in range(B):
            pc += 1
            p_qt[b] = pc
        p_chunk = {}
        for g in range(NCHUNK):
            for pt in range(_parts_of(g)):
                pc += 1
                p_chunk[(g, pt)] = pc

        def p_last(g):
            return p_chunk[(g, _parts_of(g) - 1)]

        # asem: tanh per chunk-part; exp0 (4 pieces) after tanh(4);
        # exp1 pieces {0,1} after tanh(7,0); pieces {2,3} at the end
        ac = 0
        a_tanh = {}
        for g in range(NCHUNK):
            for pt in range(_parts_of(g)):
                ac += 1
                a_tanh[(g, pt)] = ac
                if (g, pt) == (GPT + 2, 0):
                    ac += 1
                    a_exp0 = ac
                if (g, pt) == (NCHUNK - 1, 1):
                    ac += 1
                    a_exp1a = ac
        ac += 1
        a_exp1b = ac

        # vsem: qtt0 (1), bias (2), qtt1..3 (3..5), per chunk-part adds,
        # plus woven softmax steps
        vc = 0
        vc += 1
        v_bias = vc
        vc += 1
        v_qtt = {0: vc}
        v_adds = {}
        for g in range(NCHUNK):
            if g == 2:
                for b in range(1, B):
                    vc += 1
                    v_qtt[b] = vc
            for pt in range(_parts_of(g)):
                vc += 1
                v_adds[(g, pt)] = vc
            if g == GPT + 2:
                vc += 1
                v_negmax0 = vc
            if g == NCHUNK - 1:
                vc += 1
                v_negmax1a = vc
                vc += 1
                v_out0 = vc
        vc += 1
        v_negmax1b = vc
        vc += 1
        v_out1a = vc
        vc += 1
        v_out1b = vc

        with nc.Block() as block:

            @block.sync
            def _(sync):
                for h in range(2):
                    sync.dma_start(
                        out=dT_sb[:, 2 * h:2 * (h + 1), :],
                        in_=dT[:, 2 * h:2 * (h + 1), :],
                    ).then_inc(dtsem, 16)
                sync.dma_start(out=wqh_sb[:, :, :], in_=wqh[:, :, :]).then_inc(
                    wsem, 16
                )
                sync.dma_start(out=qT_sb[0][:, :, :], in_=qT[0]).then_inc(
                    qsem[0], 16
                )

                # tile 0 full, then tile 1 in two half-height pieces
                sync.wait_ge(vsem, v_out0)
                sync.dma_start(out=raw[0:128, :], in_=outt[0][:, :]).then_inc(
                    osem, 16
                )
                sync.wait_ge(vsem, v_out1a)
                sync.dma_start(out=raw[128:192, :], in_=outt[1][0:64, :]).then_inc(
                    osem, 16
                )
                sync.wait_ge(vsem, v_out1b)
                sync.dma_start(out=raw[192:256, :], in_=outt[1][64:128, :]).then_inc(
                    osem, 16
                )
                sync.wait_ge(osem, 48)

            @block.gpsimd
            def _(gpsimd):
                gpsimd.dma_start(out=bqh_sb[:, :], in_=bqh[:, :]).then_inc(
                    bqsem, 16
                )
                for b in range(1, B):
                    gpsimd.dma_start(
                        out=qT_sb[b][:, :, :], in_=qT[b]
                    ).then_inc(qsem[b], 16)
                gpsimd.dma_start(
                    out=w2oh[:, :, :], in_=w2oh_in[:, :, :]
                ).then_inc(w2sem, 16)

            @block.tensor
            def _(tensor):
                def qt_mm(b):
                    tensor.wait_ge(qsem[b], 16)
                    if b >= 2:
                        tensor.wait_ge(vsem, v_qtt[b - 2])
                    for j in range(NQC):
                        ins = nc.tensor.matmul(
                            qt_ps[b % 2][:, :],
                            wqh_sb[:, j, :],
                            qT_sb[b][:, j, :],
                            start=(j == 0),
                            stop=(j == NQC - 1),
                        )
                    ins.then_inc(psem, 1)

                tensor.wait_ge(wsem, 16)
                tensor.wait_ge(dtsem, 32)
                for j in range(NDC):
                    ins = nc.tensor.matmul(
                        ht_ps[:, :NAB],
                        wqh_sb[:, NQC + j, :],
                        dT_sb[:, j, :],
                        start=(j == 0),
                        stop=(j == NDC - 1),
                    )
                ins.then_inc(psem, 1)
                for b in range(B):
                    qt_mm(b)
                tensor.wait_ge(w2sem, 16)
                for g in range(NCHUNK):
                    t, gt = divmod(g, GPT)
                    if t == 1 and gt == 0:
                        # tile 1 reuses the (dead) prep banks
                        tensor.wait_ge(vsem, v_qtt[B - 1])
                    for pt in range(_parts_of(g)):
                        tensor.wait_ge(asem, a_tanh[(g, pt)])
                        for k in _part_ks(g, pt):
                            j, v, st, sp = _mm_plan(g, k)
                            bk = banks[j] if t == 0 else banks1[j]
                            ins = nc.tensor.matmul(
                                bk[32 * j:32 * (j + 1), :],
                                w2oh[:, v, :],
                                tin[g % 3][:, k * L_Q:(k + 1) * L_Q],
                                start=st,
                                stop=sp,
                                tile_position=(0, 32 * j),
                            )
                        ins.then_inc(psem, 1)

            @block.scalar
            def _(scalar):
                def exp_piece(t, j):
                    bk = banks[j] if t == 0 else banks1[j]
                    return nc.scalar.activation(
                        probs[t][32 * j:32 * (j + 1), :],
                        bk[32 * j:32 * (j + 1), :],
                        AF.Exp,
                        bias=negmax[t][32 * j:32 * (j + 1), :],
                        accum_out=sumexp[t][32 * j:32 * (j + 1), :],
                    )

                for g in range(NCHUNK):
                    for pt in range(_parts_of(g)):
                        scalar.wait_ge(vsem, v_adds[(g, pt)])
                        n = _parts_of(g)
                        w = (CH // n) * L_Q
                        nc.scalar.activation(
                            tin[g % 3][:, pt * w:(pt + 1) * w],
                            tin[g % 3][:, pt * w:(pt + 1) * w],
                            AF.Tanh,
                        ).then_inc(asem, 1)
                        if (g, pt) == (GPT + 2, 0):
                            scalar.wait_ge(vsem, v_negmax0)
                            nc.scalar.activation(
                                probs[0][:, :],
                                sc0[:, :],
                                AF.Exp,
                                bias=negmax[0][:, :],
                                accum_out=sumexp[0][:, :],
                            ).then_inc(asem, 1)
                        if (g, pt) == (NCHUNK - 1, 1):
                            # groups 0,1 of tile 1 completed at chunk 6
                            scalar.wait_ge(vsem, v_negmax1a)
                            for j in range(2):
                                ins = exp_piece(1, j)
                            ins.then_inc(asem, 1)
                scalar.wait_ge(psem, p_last(NCHUNK - 1))
                scalar.wait_ge(vsem, v_negmax1b)
                for j in range(2, 4):
                    ins = exp_piece(1, j)
                ins.then_inc(asem, 1)

            @block.vector
            def _(vector):
                def negmax_piece(t, j):
                    bk = banks[j] if t == 0 else banks1[j]
                    return nc.vector.tensor_reduce(
                        negmax[t][32 * j:32 * (j + 1), :],
                        bk[32 * j:32 * (j + 1), :],
                        axis=AX.X, op=ALU.max, negate=True,
                    )

                def scale_rows(t, lo, hi):
                    nc.vector.reciprocal(
                        rsum[t][lo:hi, :], sumexp[t][lo:hi, :]
                    )
                    vector.drain()
                    return nc.vector.tensor_scalar_mul(
                        outt[t][lo:hi, :], probs[t][lo:hi, :], rsum[t][lo:hi, :]
                    )

                vector.wait_ge(psem, p_ht)
                vector.wait_ge(bqsem, 16)
                nc.vector.tensor_scalar_add(
                    biasc[:, :], ht_ps[:, :NAB], bqh_sb[:, :]
                ).then_inc(vsem, 1)
                vector.wait_ge(psem, p_qt[0])
                nc.vector.tensor_copy(qtt[0][:, :], qt_ps[0][:, :]).then_inc(
                    vsem, 1
                )
                vector.drain()
                for g in range(NCHUNK):
                    if g == 2:
                        for b in range(1, B):
                            vector.wait_ge(psem, p_qt[b])
                            nc.vector.tensor_copy(
                                qtt[b][:, :], qt_ps[b % 2][:, :]
                            ).then_inc(vsem, 1)
                    b = g // CPB
                    for pt in range(_parts_of(g)):
                        if g >= 3:
                            vector.wait_ge(psem, p_last(g - 3))
                        for k in _part_ks(g, pt):
                            ci = b * A_PER + (g % CPB) * CH + k
                            ins = nc.vector.tensor_scalar_add(
                                tin[g % 3][:, k * L_Q:(k + 1) * L_Q],
                                qtt[b][:, :],
                                biasc[:, ci:ci + 1],
                            )
                        ins.then_inc(vsem, 1)
                    if g == GPT + 1:
                        vector.wait_ge(psem, p_last(GPT - 1))
                        for j in range(2):
                            nc.vector.tensor_copy(
                                sc0[:, :][32 * j:32 * (j + 1), :],
                                banks[j][32 * j:32 * (j + 1), :],
                            )
                    if g == GPT + 2:
                        for j in range(2, 4):
                            nc.vector.tensor_copy(
                                sc0[:, :][32 * j:32 * (j + 1), :],
                                banks[j][32 * j:32 * (j + 1), :],
                            )
                        vector.drain()
                        nc.vector.tensor_reduce(
                            negmax[0][:, :], sc0[:, :],
                            axis=AX.X, op=ALU.max, negate=True,
                        ).then_inc(vsem, 1)
                    if g == NCHUNK - 1:
                        # tile-1 groups 0,1 complete after chunk 5
                        vector.wait_ge(psem, p_last(NCHUNK - 3))
                        for j in range(2):
                            ins = negmax_piece(1, j)
                        ins.then_inc(vsem, 1)
                        vector.wait_ge(asem, a_exp0)
                        scale_rows(0, 0, 128).then_inc(vsem, 1)
                vector.wait_ge(psem, p_last(NCHUNK - 1))
                for j in range(2, 4):
                    ins = negmax_piece(1, j)
                ins.then_inc(vsem, 1)
                vector.wait_ge(asem, a_exp1a)
                scale_rows(1, 0, 64).then_inc(vsem, 1)
                vector.wait_ge(asem, a_exp1b)
                scale_rows(1, 64, 128).then_inc(vsem, 1)

    return nc


def _get_program():
    if "nc" not in _CACHE:
        _CACHE["nc"] = build_program()
    return _CACHE["nc"]


def _pmajor(a, nchunks):
    """(nchunks*128, X) -> (128, nchunks, X) partition-major layout."""
    x = a.reshape(nchunks, 128, a.shape[-1])
    return np.ascontiguousarray(x.transpose(1, 0, 2))


def _make_in_maps(inputs):
    import ml_dtypes

    query = np.asarray(inputs["query"], dtype=np.float32)
    decoder_states = np.asarray(inputs["decoder_states"], dtype=np.float32)
    Wq = np.asarray(inputs["Wq"], dtype=np.float32)
    Wh = np.asarray(inputs["Wh"], dtype=np.float32)
    wqh = _pmajor(np.vstack([Wq, Wh]), NWC)
    w2v = np.asarray(inputs["w2"], np.float32).reshape(H)
    w2oh = np.zeros((H, 32, 32), dtype=np.float32)
    w2oh[:, np.arange(32), np.arange(32)] = w2v[:, None]
    w2oh = w2oh.astype(ml_dtypes.bfloat16)
    bqh = np.ascontiguousarray(
        (np.asarray(inputs["bq"], np.float32)
         + np.asarray(inputs["bh"], np.float32)).reshape(H, 1)
    )
    qTf = query.transpose(1, 2, 0)  # (B, Q, L_q)
    qT = np.stack([_pmajor(qTf[b], NQC) for b in range(B)])
    in_maps = []
    for c in range(N_CORES):
        dslice = decoder_states[c * A_PER:(c + 1) * A_PER]
        # (D, B*A): column (b*A + a) holds decoder_states[a, b, :]
        dT = _pmajor(
            dslice.transpose(2, 1, 0).reshape(D_SIZE, NAB), NDC
        )
        in_maps.append({
            "qT": qT,
            "dT": dT,
            "wqh": wqh,
            "w2oh": w2oh,
            "bqh": bqh,
        })
    return in_maps


def kernel(query, decoder_states, query_mask, Wq, bq, Wh, bh, w2, b2):
    from concourse.bass_utils import run_bass_kernel_spmd

    mask = np.asarray(query_mask)
    nc = _get_program()
    in_maps = _make_in_maps({
        "query": query, "decoder_states": decoder_states,
        "Wq": Wq, "Wh": Wh, "w2": w2, "bq": bq, "bh": bh,
    })
    res = run_bass_kernel_spmd(nc, in_maps, list(range(N_CORES))).results
    perm = _row_perm()  # (A_PER, B) -> raw row
    out = np.empty((L_A, B, L_Q), dtype=np.float32)
    for c in range(N_CORES):
        out[c * A_PER:(c + 1) * A_PER] = res[c]["raw"][perm, :]

    if not mask.all():
        # exact post-exp masking + renormalization, host-side
        m = mask.T.astype(np.float32)  # (B, L_q)
        out = out * m[None, :, :]
        out = out / out.sum(axis=-1, keepdims=True)
    return out


# revision 25
# speedup vs baseline: 1.0218x; 1.0218x over previous
"""PointerNet attention scoring kernel for Trainium2 (8 NeuronCores).

Computes, for full inputs:
    q_t = query @ Wq + bq                      # (L_q, B, H)
    h_t = decoder_states @ Wh + bh             # (L_a, B, H)
    s[a,q,b] = sum_h tanh(q_t[q,b,h] + h_t[a,b,h]) * w2[h] (+ b2)
    out[a,b,q] = softmax_q(s[a,q,b])  (mask applied post-exp; ones here)

Sharding: data-parallel over L_a (512 -> 8 x 64). Each core receives the
full (host-pre-arranged, partition-major) query / weights and its
decoder_states slice, and produces a row-permuted (256, 512) block that
the host scatters into the (64, B, L_q) output slice. b2 is dropped
(softmax-invariant); the query mask, if not all ones, is applied
host-side (exactly). Host prep is layout-only - all FLOPs stay on
device.

Per-core on-chip pipeline (raw Bass, explicit semaphores - the walrus
build here only accepts one embedded sync-wait per instruction, so Tile
is unusable and all cross-engine waits are standalone wait_ge):
  - H=128 on partitions. q_tT[h,q] per b and bias columns
    h_tT[h,(b,a)]+bq+bh from small fp32 PE matmuls; stored bf16/f32.
  - Main loop, 8 chunks of CH=32 (a,b) pairs (first/last chunk split in
    two for pipeline ramp), bf16 datapath: DVE tensor_scalar_add
    broadcasts a bias column over q; one in-place ScalarE Tanh per
    chunk-part (ScalarE is the roofline: 16.8M elems / 128 lanes /
    1.2 GHz ~= 109 us); PE reduces each pair with a one-hot-scaled bf16
    w2 stationary ([128,32], w2 in column v) at tile_position (0,32j),
    accumulating into PSUM partition 32j+v of per-column-group banks
    (the 31 zero stationary columns add exact +0.0; bf16 matvecs are
    single-pass where fp32 would be two).
  - Scores tile 0 interleaves its matvecs over all 4 column-groups
    (4-way PE concurrency); its softmax hides under the next tanh.
    Tile 1 fills groups {0,1} during chunks 4-5 and {2,3} during 6-7
    (2-way concurrency) so half its softmax also hides under tanh and
    only groups 2,3 drain at the kernel tail.
  - Softmax over q (free axis), fp32, per 32-row bank piece: DVE
    negated max, ScalarE Exp with bias=-max and fused row-sum accum,
    DVE reciprocal + scale, 128/256 KB output DMAs.
"""

import numpy as np

L_Q, L_A, B = 512, 512, 4
Q_SIZE, D_SIZE, H = 256, 512, 128
N_CORES = 8
A_PER = L_A // N_CORES  # 64
CH = 32                 # (a,b) pairs per tanh chunk
NCHUNK = (A_PER * B) // CH          # 8
NTILE = (A_PER * B) // 128          # 2 scores tiles of 128 pair-rows
NAB = A_PER * B                     # 256 pair rows
GPT = 128 // CH                     # 4 chunks per scores tile
CPB = A_PER // CH                   # 2 chunks per batch entry
NQC = Q_SIZE // 128                 # 2 contraction chunks for q_t
NDC = D_SIZE // 128                 # 4 contraction chunks for h_t
NWC = NQC + NDC

_CACHE = {}


def _parts_of(g):
    return 4 if g in (0, NCHUNK - 1) else 1


def _part_ks(g, pt):
    n = _parts_of(g)
    lo = pt * (CH // n)
    return range(lo, lo + CH // n)


def _mm_plan(g, k):
    """(bank j, one-hot column v, start, stop) for pair-block k of chunk g."""
    t, gt = divmod(g, GPT)
    if t == 0:
        j = k % 4
        v = 8 * gt + k // 4
        return j, v, (gt == 0 and k < 4), (gt == GPT - 1 and k >= CH - 4)
    j = 2 * (gt // 2) + k % 2
    v = 16 * (gt % 2) + k // 2
    return j, v, (gt % 2 == 0 and k < 2), (gt % 2 == 1 and k >= CH - 2)


def _row_perm():
    """perm[a, b] = raw row index holding out[a, b, :]."""
    perm = np.empty((A_PER, B), dtype=np.int64)
    for g in range(NCHUNK):
        t = g // GPT
        b = g // CPB
        for k in range(CH):
            a = (g % CPB) * CH + k
            j, v, _, _ = _mm_plan(g, k)
            perm[a, b] = t * 128 + 32 * j + v
    return perm


def build_program():
    from contextlib import ExitStack

    import concourse.bass as bass
    from concourse import mybir

    f32 = mybir.dt.float32
    bf16 = mybir.dt.bfloat16
    AF = mybir.ActivationFunctionType
    ALU = mybir.AluOpType
    AX = mybir.AxisListType

    nc = bass.Bass()
    qT = nc.declare_dram_parameter("qT", [B, 128, NQC, L_Q], f32, isOutput=False)
    dT = nc.declare_dram_parameter("dT", [128, NDC, NAB], f32, isOutput=False)
    wqh = nc.declare_dram_parameter("wqh", [128, NWC, H], f32, isOutput=False)
    w2oh_in = nc.declare_dram_parameter("w2oh", [H, 32, 32], bf16, isOutput=False)
    bqh = nc.declare_dram_parameter("bqh", [H, 1], f32, isOutput=False)
    raw = nc.declare_dram_parameter("raw", [NAB, L_Q], f32, isOutput=True)

    with ExitStack() as ctx:
        _n = [0]

        def sb(shape, dt=f32):
            _n[0] += 1
            return ctx.enter_context(nc.sbuf_tensor(f"sb{_n[0]}", shape, dt))

        def ps(shape):
            _n[0] += 1
            return ctx.enter_context(nc.psum_tensor(f"ps{_n[0]}", shape, f32))

        wqh_sb = sb([128, NWC, H])
        w2oh = sb([128, 32, 32], bf16)
        bqh_sb = sb([128, 1])
        qT_sb = [sb([128, NQC, L_Q]) for _ in range(B)]
        dT_sb = sb([128, NDC, NAB])
        qtt = [sb([128, L_Q], bf16) for _ in range(B)]
        biasc = sb([128, NAB])  # fp32: tensor_scalar scalar1 must be f32
        tin = [sb([128, CH * L_Q], bf16) for _ in range(3)]
        probs = [sb([128, L_Q]) for _ in range(NTILE)]
        outt = [sb([128, L_Q]) for _ in range(NTILE)]
        sc0 = sb([128, L_Q])  # tile-0 scores gathered from the 4 banks
        negmax = [sb([128, 1]) for _ in range(NTILE)]
        sumexp = [sb([128, 1]) for _ in range(NTILE)]
        rsum = [sb([128, 1]) for _ in range(NTILE)]

        qt_ps = [ps([128, L_Q]) for _ in range(2)]
        ht_ps = ps([128, L_Q])   # prep uses [:, :NAB]; later a score bank
        spare_ps = ps([128, L_Q])
        banks = [ps([128, L_Q]) for _ in range(4)]  # tile-0 col-group scores
        banks1 = [qt_ps[0], qt_ps[1], ht_ps, spare_ps]  # tile-1 reuses prep

        wsem = ctx.enter_context(nc.semaphore("wsem"))
        qsem = [ctx.enter_context(nc.semaphore(f"qsem{b}")) for b in range(B)]
        dtsem = ctx.enter_context(nc.semaphore("dtsem"))
        bqsem = ctx.enter_context(nc.semaphore("bqsem"))
        w2sem = ctx.enter_context(nc.semaphore("w2sem"))
        psem = ctx.enter_context(nc.semaphore("psem"))
        asem = ctx.enter_context(nc.semaphore("asem"))
        vsem = ctx.enter_context(nc.semaphore("vsem"))
        osem = ctx.enter_context(nc.semaphore("osem"))

        # --- semaphore milestones (mirror each engine's program order)
        # psem: qt b0 (1), ht (2), qt b1..b3 (3..5), then per chunk-part
        pc = 0
        pc += 1
        p_ht = pc
        p_qt = {}
        for b in range(B):
            pc += 1
            p_qt[b] = pc
        p_chunk = {}
        for g in range(NCHUNK):
            for pt in range(_parts_of(g)):
                pc += 1
                p_chunk[(g, pt)] = pc

        def p_last(g):
            return p_chunk[(g, _parts_of(g) - 1)]

        # asem: tanh per chunk-part; exp0 (4 pieces) after tanh(4);
        # exp1 pieces {0,1} after tanh(7,0); pieces {2,3} at the end
        ac = 0
        a_tanh = {}
        for g in range(NCHUNK):
            for pt in range(_parts_of(g)):
                ac += 1
                a_tanh[(g, pt)] = ac
                if (g, pt) == (GPT + 2, 0):
                    ac += 1
                    a_exp0 = ac
                if (g, pt) == (NCHUNK - 1, 1):
                    ac += 1
                    a_exp1a = ac
        ac += 1
        a_exp1b = ac

        # vsem: qtt0 (1), bias (2), qtt1..3 (3..5), per chunk-part adds,
        # plus woven softmax steps
        vc = 0
        vc += 1
        v_bias = vc
        vc += 1
        v_qtt = {0: vc}
        v_adds = {}
        for g in range(NCHUNK):
            if g == 2:
                for b in range(1, B):
                    vc += 1
                    v_qtt[b] = vc
            for pt in range(_parts_of(g)):
                vc += 1
                v_adds[(g, pt)] = vc
            if g == GPT + 2:
                vc += 1
                v_negmax0 = vc
            if g == NCHUNK - 1:
                vc += 1
                v_negmax1a = vc
                vc += 1
                v_out0 = vc
        vc += 1
        v_negmax1b = vc
        vc += 1
        v_out1a = vc
        vc += 1
        v_out1b = vc

        with nc.Block() as block:

            @block.sync
            def _(sync):
                for h in range(2):
                    sync.dma_start(
                        out=dT_sb[:, 2 * h:2 * (h + 1), :],
                        in_=dT[:, 2 * h:2 * (h + 1), :],
                    ).then_inc(dtsem, 16)
                sync.dma_start(out=wqh_sb[:, :, :], in_=wqh[:, :, :]).then_inc(
                    wsem, 16
                )
                sync.dma_start(out=qT_sb[0][:, :, :], in_=qT[0]).then_inc(
                    qsem[0], 16
                )

                # tile 0 full, then tile 1 in two half-height pieces
                sync.wait_ge(vsem, v_out0)
                sync.dma_start(out=raw[0:128, :], in_=outt[0][:, :]).then_inc(
                    osem, 16
                )
                sync.wait_ge(vsem, v_out1a)
                sync.dma_start(out=raw[128:192, :], in_=outt[1][0:64, :]).then_inc(
                    osem, 16
                )
                sync.wait_ge(vsem, v_out1b)
                sync.dma_start(out=raw[192:256, :], in_=outt[1][64:128, :]).then_inc(
                    osem, 16
                )
                sync.wait_ge(osem, 48)

            @block.gpsimd
            def _(gpsimd):
                gpsimd.dma_start(out=bqh_sb[:, :], in_=bqh[:, :]).then_inc(
                    bqsem, 16
                )
                # keep the bulk secondary loads off the critical dT/qT0 path
                gpsimd.wait_ge(qsem[0], 16)
                for b in range(1, B):
                    gpsimd.dma_start(
                        out=qT_sb[b][:, :, :], in_=qT[b]
                    ).then_inc(qsem[b], 16)
                gpsimd.dma_start(
                    out=w2oh[:, :, :], in_=w2oh_in[:, :, :]
                ).then_inc(w2sem, 16)

            @block.tensor
            def _(tensor):
                def qt_mm(b):
                    tensor.wait_ge(qsem[b], 16)
                    if b >= 2:
                        tensor.wait_ge(vsem, v_qtt[b - 2])
                    for j in range(NQC):
                        ins = nc.tensor.matmul(
                            qt_ps[b % 2][:, :],
                            wqh_sb[:, j, :],
                            qT_sb[b][:, j, :],
                            start=(j == 0),
                            stop=(j == NQC - 1),
                        )
                    ins.then_inc(psem, 1)

                tensor.wait_ge(wsem, 16)
                tensor.wait_ge(dtsem, 32)
                for j in range(NDC):
                    ins = nc.tensor.matmul(
                        ht_ps[:, :NAB],
                        wqh_sb[:, NQC + j, :],
                        dT_sb[:, j, :],
                        start=(j == 0),
                        stop=(j == NDC - 1),
                    )
                ins.then_inc(psem, 1)
                for b in range(B):
                    qt_mm(b)
                tensor.wait_ge(w2sem, 16)
                for g in range(NCHUNK):
                    t, gt = divmod(g, GPT)
                    if t == 1 and gt == 0:
                        # tile 1 reuses the (dead) prep banks
                        tensor.wait_ge(vsem, v_qtt[B - 1])
                    for pt in range(_parts_of(g)):
                        tensor.wait_ge(asem, a_tanh[(g, pt)])
                        for k in _part_ks(g, pt):
                            j, v, st, sp = _mm_plan(g, k)
                            bk = banks[j] if t == 0 else banks1[j]
                            ins = nc.tensor.matmul(
                                bk[32 * j:32 * (j + 1), :],
                                w2oh[:, v, :],
                                tin[g % 3][:, k * L_Q:(k + 1) * L_Q],
                                start=st,
                                stop=sp,
                                tile_position=(0, 32 * j),
                            )
                        ins.then_inc(psem, 1)

            @block.scalar
            def _(scalar):
                def exp_piece(t, j):
                    bk = banks[j] if t == 0 else banks1[j]
                    return nc.scalar.activation(
                        probs[t][32 * j:32 * (j + 1), :],
                        bk[32 * j:32 * (j + 1), :],
                        AF.Exp,
                        bias=negmax[t][32 * j:32 * (j + 1), :],
                        accum_out=sumexp[t][32 * j:32 * (j + 1), :],
                    )

                for g in range(NCHUNK):
                    for pt in range(_parts_of(g)):
                        scalar.wait_ge(vsem, v_adds[(g, pt)])
                        n = _parts_of(g)
                        w = (CH // n) * L_Q
                        nc.scalar.activation(
                            tin[g % 3][:, pt * w:(pt + 1) * w],
                            tin[g % 3][:, pt * w:(pt + 1) * w],
                            AF.Tanh,
                        ).then_inc(asem, 1)
                        if (g, pt) == (GPT + 2, 0):
                            scalar.wait_ge(vsem, v_negmax0)
                            nc.scalar.activation(
                                probs[0][:, :],
                                sc0[:, :],
                                AF.Exp,
                                bias=negmax[0][:, :],
                                accum_out=sumexp[0][:, :],
                            ).then_inc(asem, 1)
                        if (g, pt) == (NCHUNK - 1, 1):
                            # groups 0,1 of tile 1 completed at chunk 6
                            scalar.wait_ge(vsem, v_negmax1a)
                            for j in range(2):
                                ins = exp_piece(1, j)
                            ins.then_inc(asem, 1)
                scalar.wait_ge(psem, p_last(NCHUNK - 1))
                scalar.wait_ge(vsem, v_negmax1b)
                for j in range(2, 4):
                    ins = exp_piece(1, j)
                ins.then_inc(asem, 1)

            @block.vector
            def _(vector):
                def negmax_piece(t, j):
                    bk = banks[j] if t == 0 else banks1[j]
                    return nc.vector.tensor_reduce(
                        negmax[t][32 * j:32 * (j + 1), :],
                        bk[32 * j:32 * (j + 1), :],
                        axis=AX.X, op=ALU.max, negate=True,
                    )

                def scale_rows(t, lo, hi):
                    nc.vector.reciprocal(
                        rsum[t][lo:hi, :], sumexp[t][lo:hi, :]
                    )
                    vector.drain()
                    return nc.vector.tensor_scalar_mul(
                        outt[t][lo:hi, :], probs[t][lo:hi, :], rsum[t][lo:hi, :]
                    )

                vector.wait_ge(psem, p_ht)
                vector.wait_ge(bqsem, 16)
                nc.vector.tensor_scalar_add(
                    biasc[:, :], ht_ps[:, :NAB], bqh_sb[:, :]
                ).then_inc(vsem, 1)
                vector.wait_ge(psem, p_qt[0])
                nc.vector.tensor_copy(qtt[0][:, :], qt_ps[0][:, :]).then_inc(
                    vsem, 1
                )
                vector.drain()
                for g in range(NCHUNK):
                    if g == 2:
                        for b in range(1, B):
                            vector.wait_ge(psem, p_qt[b])
                            nc.vector.tensor_copy(
                                qtt[b][:, :], qt_ps[b % 2][:, :]
                            ).then_inc(vsem, 1)
                    b = g // CPB
                    for pt in range(_parts_of(g)):
                        if g >= 3:
                            vector.wait_ge(psem, p_last(g - 3))
                        for k in _part_ks(g, pt):
                            ci = b * A_PER + (g % CPB) * CH + k
                            ins = nc.vector.tensor_scalar_add(
                                tin[g % 3][:, k * L_Q:(k + 1) * L_Q],
                                qtt[b][:, :],
                                biasc[:, ci:ci + 1],
                            )
                        ins.then_inc(vsem, 1)
                    if g == GPT + 1:
                        vector.wait_ge(psem, p_last(GPT - 1))
                        for j in range(2):
                            nc.vector.tensor_copy(
                                sc0[:, :][32 * j:32 * (j + 1), :],
                                banks[j][32 * j:32 * (j + 1), :],
                            )
                    if g == GPT + 2:
                        for j in range(2, 4):
                            nc.vector.tensor_copy(
                                sc0[:, :][32 * j:32 * (j + 1), :],
                                banks[j][32 * j:32 * (j + 1), :],
                            )
                        vector.drain()
                        nc.vector.tensor_reduce(
                            negmax[0][:, :], sc0[:, :],
                            axis=AX.X, op=ALU.max, negate=True,
                        ).then_inc(vsem, 1)
                    if g == NCHUNK - 1:
                        # tile-1 groups 0,1 complete after chunk 5
                        vector.wait_ge(psem, p_last(NCHUNK - 3))
                        for j in range(2):
                            ins = negmax_piece(1, j)
                        ins.then_inc(vsem, 1)
                        vector.wait_ge(asem, a_exp0)
                        scale_rows(0, 0, 128).then_inc(vsem, 1)
                vector.wait_ge(psem, p_last(NCHUNK - 1))
                for j in range(2, 4):
                    ins = negmax_piece(1, j)
                ins.then_inc(vsem, 1)
                vector.wait_ge(asem, a_exp1a)
                scale_rows(1, 0, 64).then_inc(vsem, 1)
                vector.wait_ge(asem, a_exp1b)
                scale_rows(1, 64, 128).then_inc(vsem, 1)

    return nc


def _get_program():
    if "nc" not in _CACHE:
        _CACHE["nc"] = build_program()
    return _CACHE["nc"]


def _pmajor(a, nchunks):
    """(nchunks*128, X) -> (128, nchunks, X) partition-major layout."""
    x = a.reshape(nchunks, 128, a.shape[-1])
    return np.ascontiguousarray(x.transpose(1, 0, 2))


def _make_in_maps(inputs):
    import ml_dtypes

    query = np.asarray(inputs["query"], dtype=np.float32)
    decoder_states = np.asarray(inputs["decoder_states"], dtype=np.float32)
    Wq = np.asarray(inputs["Wq"], dtype=np.float32)
    Wh = np.asarray(inputs["Wh"], dtype=np.float32)
    wqh = _pmajor(np.vstack([Wq, Wh]), NWC)
    w2v = np.asarray(inputs["w2"], np.float32).reshape(H)
    w2oh = np.zeros((H, 32, 32), dtype=np.float32)
    w2oh[:, np.arange(32), np.arange(32)] = w2v[:, None]
    w2oh = w2oh.astype(ml_dtypes.bfloat16)
    bqh = np.ascontiguousarray(
        (np.asarray(inputs["bq"], np.float32)
         + np.asarray(inputs["bh"], np.float32)).reshape(H, 1)
    )
    qTf = query.transpose(1, 2, 0)  # (B, Q, L_q)
    qT = np.stack([_pmajor(qTf[b], NQC) for b in range(B)])
    in_maps = []
    for c in range(N_CORES):
        dslice = decoder_states[c * A_PER:(c + 1) * A_PER]
        # (D, B*A): column (b*A + a) holds decoder_states[a, b, :]
        dT = _pmajor(
            dslice.transpose(2, 1, 0).reshape(D_SIZE, NAB), NDC
        )
        in_maps.append({
            "qT": qT,
            "dT": dT,
            "wqh": wqh,
            "w2oh": w2oh,
            "bqh": bqh,
        })
    return in_maps


def kernel(query, decoder_states, query_mask, Wq, bq, Wh, bh, w2, b2):
    from concourse.bass_utils import run_bass_kernel_spmd

    mask = np.asarray(query_mask)
    nc = _get_program()
    in_maps = _make_in_maps({
        "query": query, "decoder_states": decoder_states,
        "Wq": Wq, "Wh": Wh, "w2": w2, "bq": bq, "bh": bh,
    })
    res = run_bass_kernel_spmd(nc, in_maps, list(range(N_CORES))).results
    perm = _row_perm()  # (A_PER, B) -> raw row
    out = np.empty((L_A, B, L_Q), dtype=np.float32)
    for c in range(N_CORES):
        out[c * A_PER:(c + 1) * A_PER] = res[c]["raw"][perm, :]

    if not mask.all():
        # exact post-exp masking + renormalization, host-side
        m = mask.T.astype(np.float32)  # (B, L_q)
        out = out * m[None, :, :]
        out = out / out.sum(axis=-1, keepdims=True)
    return out


# revision 26
# speedup vs baseline: 1.0322x; 1.0101x over previous
"""PointerNet attention scoring kernel for Trainium2 (8 NeuronCores).

Computes, for full inputs:
    q_t = query @ Wq + bq                      # (L_q, B, H)
    h_t = decoder_states @ Wh + bh             # (L_a, B, H)
    s[a,q,b] = sum_h tanh(q_t[q,b,h] + h_t[a,b,h]) * w2[h] (+ b2)
    out[a,b,q] = softmax_q(s[a,q,b])  (mask applied post-exp; ones here)

Sharding: data-parallel over L_a (512 -> 8 x 64). Each core receives the
full (host-pre-arranged, partition-major) query / weights and its
decoder_states slice, and produces a row-permuted (256, 512) block that
the host scatters into the (64, B, L_q) output slice. b2 is dropped
(softmax-invariant); the query mask, if not all ones, is applied
host-side (exactly). Host prep is layout-only - all FLOPs stay on
device.

Per-core on-chip pipeline (raw Bass, explicit semaphores - the walrus
build here only accepts one embedded sync-wait per instruction, so Tile
is unusable and all cross-engine waits are standalone wait_ge):
  - H=128 on partitions. q_tT[h,q] per b and bias columns
    h_tT[h,(b,a)]+bq+bh from small fp32 PE matmuls; stored bf16/f32.
  - Main loop, 8 chunks of CH=32 (a,b) pairs (first/last chunk split in
    two for pipeline ramp), bf16 datapath: DVE tensor_scalar_add
    broadcasts a bias column over q; one in-place ScalarE Tanh per
    chunk-part (ScalarE is the roofline: 16.8M elems / 128 lanes /
    1.2 GHz ~= 109 us); PE reduces each pair with a one-hot-scaled bf16
    w2 stationary ([128,32], w2 in column v) at tile_position (0,32j),
    accumulating into PSUM partition 32j+v of per-column-group banks
    (the 31 zero stationary columns add exact +0.0; bf16 matvecs are
    single-pass where fp32 would be two).
  - Scores tile 0 interleaves its matvecs over all 4 column-groups
    (4-way PE concurrency); its softmax hides under the next tanh.
    Tile 1 fills groups {0,1} during chunks 4-5 and {2,3} during 6-7
    (2-way concurrency) so half its softmax also hides under tanh and
    only groups 2,3 drain at the kernel tail.
  - Softmax over q (free axis), fp32, per 32-row bank piece: DVE
    negated max, ScalarE Exp with bias=-max and fused row-sum accum,
    DVE reciprocal + scale, 128/256 KB output DMAs.
"""

import numpy as np

L_Q, L_A, B = 512, 512, 4
Q_SIZE, D_SIZE, H = 256, 512, 128
N_CORES = 8
A_PER = L_A // N_CORES  # 64
CH = 32                 # (a,b) pairs per tanh chunk
NCHUNK = (A_PER * B) // CH          # 8
NTILE = (A_PER * B) // 128          # 2 scores tiles of 128 pair-rows
NAB = A_PER * B                     # 256 pair rows
GPT = 128 // CH                     # 4 chunks per scores tile
CPB = A_PER // CH                   # 2 chunks per batch entry
NQC = Q_SIZE // 128                 # 2 contraction chunks for q_t
NDC = D_SIZE // 128                 # 4 contraction chunks for h_t
NWC = NQC + NDC

_CACHE = {}


def _parts_of(g):
    return 4 if g in (0, NCHUNK - 1) else 1


def _part_ks(g, pt):
    n = _parts_of(g)
    lo = pt * (CH // n)
    return range(lo, lo + CH // n)


def _mm_plan(g, k):
    """(bank j, one-hot column v, start, stop) for pair-block k of chunk g."""
    t, gt = divmod(g, GPT)
    if t == 0:
        j = k % 4
        v = 8 * gt + k // 4
        return j, v, (gt == 0 and k < 4), (gt == GPT - 1 and k >= CH - 4)
    j = 2 * (gt // 2) + k % 2
    v = 16 * (gt % 2) + k // 2
    return j, v, (gt % 2 == 0 and k < 2), (gt % 2 == 1 and k >= CH - 2)


def _row_perm():
    """perm[a, b] = raw row index holding out[a, b, :]."""
    perm = np.empty((A_PER, B), dtype=np.int64)
    for g in range(NCHUNK):
        t = g // GPT
        b = g // CPB
        for k in range(CH):
            a = (g % CPB) * CH + k
            j, v, _, _ = _mm_plan(g, k)
            perm[a, b] = t * 128 + 32 * j + v
    return perm


def build_program():
    from contextlib import ExitStack

    import concourse.bass as bass
    from concourse import mybir

    f32 = mybir.dt.float32
    bf16 = mybir.dt.bfloat16
    AF = mybir.ActivationFunctionType
    ALU = mybir.AluOpType
    AX = mybir.AxisListType

    nc = bass.Bass()
    qT = nc.declare_dram_parameter("qT", [B, 128, NQC, L_Q], f32, isOutput=False)
    dT = nc.declare_dram_parameter("dT", [128, NDC, NAB], f32, isOutput=False)
    wqh = nc.declare_dram_parameter("wqh", [128, NWC, H], f32, isOutput=False)
    w2oh_in = nc.declare_dram_parameter("w2oh", [H, 32, 32], bf16, isOutput=False)
    bqh = nc.declare_dram_parameter("bqh", [H, 1], f32, isOutput=False)
    raw = nc.declare_dram_parameter("raw", [NAB, L_Q], f32, isOutput=True)

    with ExitStack() as ctx:
        _n = [0]

        def sb(shape, dt=f32):
            _n[0] += 1
            return ctx.enter_context(nc.sbuf_tensor(f"sb{_n[0]}", shape, dt))

        def ps(shape):
            _n[0] += 1
            return ctx.enter_context(nc.psum_tensor(f"ps{_n[0]}", shape, f32))

        wqh_sb = sb([128, NWC, H])
        w2oh = sb([128, 32, 32], bf16)
        bqh_sb = sb([128, 1])
        qT_sb = [sb([128, NQC, L_Q]) for _ in range(B)]
        dT_sb = sb([128, NDC, NAB])
        qtt = [sb([128, L_Q], bf16) for _ in range(B)]
        biasc = sb([128, NAB])  # fp32: tensor_scalar scalar1 must be f32
        tin = [sb([128, CH * L_Q], bf16) for _ in range(3)]
        probs = [sb([128, L_Q]) for _ in range(NTILE)]
        outt = [sb([128, L_Q]) for _ in range(NTILE)]
        sc0 = sb([128, L_Q])  # tile-0 scores gathered from the 4 banks
        negmax = [sb([128, 1]) for _ in range(NTILE)]
        sumexp = [sb([128, 1]) for _ in range(NTILE)]
        rsum = [sb([128, 1]) for _ in range(NTILE)]

        qt_ps = [ps([128, L_Q]) for _ in range(2)]
        ht_ps = ps([128, L_Q])   # prep uses [:, :NAB]; later a score bank
        spare_ps = ps([128, L_Q])
        banks = [ps([128, L_Q]) for _ in range(4)]  # tile-0 col-group scores
        banks1 = [qt_ps[0], qt_ps[1], ht_ps, spare_ps]  # tile-1 reuses prep

        wsem = ctx.enter_context(nc.semaphore("wsem"))
        qsem = [ctx.enter_context(nc.semaphore(f"qsem{b}")) for b in range(B)]
        dtsem = ctx.enter_context(nc.semaphore("dtsem"))
        bqsem = ctx.enter_context(nc.semaphore("bqsem"))
        w2sem = ctx.enter_context(nc.semaphore("w2sem"))
        psem = ctx.enter_context(nc.semaphore("psem"))
        asem = ctx.enter_context(nc.semaphore("asem"))
        vsem = ctx.enter_context(nc.semaphore("vsem"))
        osem = ctx.enter_context(nc.semaphore("osem"))

        # --- semaphore milestones (mirror each engine's program order)
        # psem: qt b0 (1), ht (2), qt b1..b3 (3..5), then per chunk-part
        pc = 0
        pc += 1
        p_ht = pc
        p_qt = {}
        for b in range(B):
            pc += 1
            p_qt[b] = pc
        p_chunk = {}
        for g in range(NCHUNK):
            for pt in range(_parts_of(g)):
                pc += 1
                p_chunk[(g, pt)] = pc

        def p_last(g):
            return p_chunk[(g, _parts_of(g) - 1)]

        # asem: tanh per chunk-part; exp0 (4 pieces) after tanh(4);
        # exp1 pieces {0,1} after tanh(7,0); pieces {2,3} at the end
        ac = 0
        a_tanh = {}
        for g in range(NCHUNK):
            for pt in range(_parts_of(g)):
                ac += 1
                a_tanh[(g, pt)] = ac
                if (g, pt) == (GPT + 2, 0):
                    ac += 1
                    a_exp0 = ac
                if (g, pt) == (NCHUNK - 1, 1):
                    ac += 1
                    a_exp1a = ac
        ac += 1
        a_exp1b = ac

        # vsem: qtt0 (1), bias (2), qtt1..3 (3..5), per chunk-part adds,
        # plus woven softmax steps
        vc = 0
        vc += 1
        v_bias = vc
        vc += 1
        v_qtt = {0: vc}
        v_adds = {}
        for g in range(NCHUNK):
            if g == 2:
                for b in range(1, B):
                    vc += 1
                    v_qtt[b] = vc
            for pt in range(_parts_of(g)):
                vc += 1
                v_adds[(g, pt)] = vc
            if g == GPT + 2:
                vc += 1
                v_negmax0 = vc
            if g == NCHUNK - 1:
                vc += 1
                v_negmax1a = vc
                vc += 1
                v_out0 = vc
        vc += 1
        v_negmax1b = vc
        vc += 1
        v_out1a = vc
        vc += 1
        v_out1b = vc

        with nc.Block() as block:

            @block.sync
            def _(sync):
                for h in range(2):
                    sync.dma_start(
                        out=dT_sb[:, 2 * h:2 * (h + 1), :],
                        in_=dT[:, 2 * h:2 * (h + 1), :],
                    ).then_inc(dtsem, 16)
                sync.dma_start(out=wqh_sb[:, :, :], in_=wqh[:, :, :]).then_inc(
                    wsem, 16
                )
                sync.dma_start(out=qT_sb[0][:, :, :], in_=qT[0]).then_inc(
                    qsem[0], 16
                )

                # tile 0 full, then tile 1 in two half-height pieces
                sync.wait_ge(vsem, v_out0)
                sync.dma_start(out=raw[0:128, :], in_=outt[0][:, :]).then_inc(
                    osem, 16
                )
                sync.wait_ge(vsem, v_out1a)
                sync.dma_start(out=raw[128:192, :], in_=outt[1][0:64, :]).then_inc(
                    osem, 16
                )
                sync.wait_ge(vsem, v_out1b)
                sync.dma_start(out=raw[192:256, :], in_=outt[1][64:128, :]).then_inc(
                    osem, 16
                )
                sync.wait_ge(osem, 48)

            @block.gpsimd
            def _(gpsimd):
                gpsimd.dma_start(out=bqh_sb[:, :], in_=bqh[:, :]).then_inc(
                    bqsem, 16
                )
                # keep the bulk secondary loads off the critical dT/qT0 path
                gpsimd.wait_ge(qsem[0], 16)
                for b in range(1, B):
                    gpsimd.dma_start(
                        out=qT_sb[b][:, :, :], in_=qT[b]
                    ).then_inc(qsem[b], 16)
                gpsimd.dma_start(
                    out=w2oh[:, :, :], in_=w2oh_in[:, :, :]
                ).then_inc(w2sem, 16)

            @block.tensor
            def _(tensor):
                def qt_mm(b):
                    tensor.wait_ge(qsem[b], 16)
                    if b >= 2:
                        tensor.wait_ge(vsem, v_qtt[b - 2])
                    for j in range(NQC):
                        ins = nc.tensor.matmul(
                            qt_ps[b % 2][:, :],
                            wqh_sb[:, j, :],
                            qT_sb[b][:, j, :],
                            start=(j == 0),
                            stop=(j == NQC - 1),
                        )
                    ins.then_inc(psem, 1)

                tensor.wait_ge(wsem, 16)
                tensor.wait_ge(dtsem, 32)
                for j in range(NDC):
                    ins = nc.tensor.matmul(
                        ht_ps[:, :NAB],
                        wqh_sb[:, NQC + j, :],
                        dT_sb[:, j, :],
                        start=(j == 0),
                        stop=(j == NDC - 1),
                    )
                ins.then_inc(psem, 1)
                for b in range(B):
                    qt_mm(b)
                tensor.wait_ge(w2sem, 16)
                for g in range(NCHUNK):
                    t, gt = divmod(g, GPT)
                    if t == 1 and gt == 0:
                        # tile 1 reuses the (dead) prep banks
                        tensor.wait_ge(vsem, v_qtt[B - 1])
                    for pt in range(_parts_of(g)):
                        tensor.wait_ge(asem, a_tanh[(g, pt)])
                        for k in _part_ks(g, pt):
                            j, v, st, sp = _mm_plan(g, k)
                            bk = banks[j] if t == 0 else banks1[j]
                            ins = nc.tensor.matmul(
                                bk[32 * j:32 * (j + 1), :],
                                w2oh[:, v, :],
                                tin[g % 3][:, k * L_Q:(k + 1) * L_Q],
                                start=st,
                                stop=sp,
                                tile_position=(0, 32 * j),
                            )
                        ins.then_inc(psem, 1)

            @block.scalar
            def _(scalar):
                def exp_piece(t, j):
                    bk = banks[j] if t == 0 else banks1[j]
                    return nc.scalar.activation(
                        probs[t][32 * j:32 * (j + 1), :],
                        bk[32 * j:32 * (j + 1), :],
                        AF.Exp,
                        bias=negmax[t][32 * j:32 * (j + 1), :],
                        accum_out=sumexp[t][32 * j:32 * (j + 1), :],
                    )

                for g in range(NCHUNK):
                    for pt in range(_parts_of(g)):
                        scalar.wait_ge(vsem, v_adds[(g, pt)])
                        n = _parts_of(g)
                        w = (CH // n) * L_Q
                        nc.scalar.activation(
                            tin[g % 3][:, pt * w:(pt + 1) * w],
                            tin[g % 3][:, pt * w:(pt + 1) * w],
                            AF.Tanh,
                        ).then_inc(asem, 1)
                        if (g, pt) == (GPT + 2, 0):
                            scalar.wait_ge(vsem, v_negmax0)
                            nc.scalar.activation(
                                probs[0][:, :],
                                sc0[:, :],
                                AF.Exp,
                                bias=negmax[0][:, :],
                                accum_out=sumexp[0][:, :],
                            ).then_inc(asem, 1)
                        if (g, pt) == (NCHUNK - 1, 1):
                            # groups 0,1 of tile 1 completed at chunk 6
                            scalar.wait_ge(vsem, v_negmax1a)
                            for j in range(2):
                                ins = exp_piece(1, j)
                            ins.then_inc(asem, 1)
                scalar.wait_ge(psem, p_last(NCHUNK - 1))
                scalar.wait_ge(vsem, v_negmax1b)
                for j in range(2, 4):
                    ins = exp_piece(1, j)
                ins.then_inc(asem, 1)

            @block.vector
            def _(vector):
                def negmax_piece(t, j):
                    bk = banks[j] if t == 0 else banks1[j]
                    return nc.vector.tensor_reduce(
                        negmax[t][32 * j:32 * (j + 1), :],
                        bk[32 * j:32 * (j + 1), :],
                        axis=AX.X, op=ALU.max, negate=True,
                    )

                def scale_rows(t, lo, hi):
                    nc.vector.reciprocal(
                        rsum[t][lo:hi, :], sumexp[t][lo:hi, :]
                    )
                    vector.drain()
                    return nc.vector.tensor_scalar_mul(
                        outt[t][lo:hi, :], probs[t][lo:hi, :], rsum[t][lo:hi, :]
                    )

                vector.wait_ge(psem, p_ht)
                vector.wait_ge(bqsem, 16)
                nc.vector.tensor_scalar_add(
                    biasc[:, :], ht_ps[:, :NAB], bqh_sb[:, :]
                ).then_inc(vsem, 1)
                vector.wait_ge(psem, p_qt[0])
                nc.vector.tensor_copy(qtt[0][:, :], qt_ps[0][:, :]).then_inc(
                    vsem, 1
                )
                vector.drain()
                for g in range(NCHUNK):
                    if g == 2:
                        for b in range(1, B):
                            vector.wait_ge(psem, p_qt[b])
                            nc.vector.tensor_copy(
                                qtt[b][:, :], qt_ps[b % 2][:, :]
                            ).then_inc(vsem, 1)
                        vector.drain()
                    b = g // CPB
                    for pt in range(_parts_of(g)):
                        if g >= 3:
                            vector.wait_ge(psem, p_last(g - 3))
                        for k in _part_ks(g, pt):
                            ci = b * A_PER + (g % CPB) * CH + k
                            ins = nc.vector.tensor_scalar_add(
                                tin[g % 3][:, k * L_Q:(k + 1) * L_Q],
                                qtt[b][:, :],
                                biasc[:, ci:ci + 1],
                            )
                        ins.then_inc(vsem, 1)
                    if g == GPT + 1:
                        vector.wait_ge(psem, p_last(GPT - 1))
                        for j in range(2):
                            nc.vector.tensor_copy(
                                sc0[:, :][32 * j:32 * (j + 1), :],
                                banks[j][32 * j:32 * (j + 1), :],
                            )
                    if g == GPT + 2:
                        for j in range(2, 4):
                            nc.vector.tensor_copy(
                                sc0[:, :][32 * j:32 * (j + 1), :],
                                banks[j][32 * j:32 * (j + 1), :],
                            )
                        vector.drain()
                        nc.vector.tensor_reduce(
                            negmax[0][:, :], sc0[:, :],
                            axis=AX.X, op=ALU.max, negate=True,
                        ).then_inc(vsem, 1)
                    if g == NCHUNK - 1:
                        # tile-1 groups 0,1 complete after chunk 5
                        vector.wait_ge(psem, p_last(NCHUNK - 3))
                        for j in range(2):
                            ins = negmax_piece(1, j)
                        ins.then_inc(vsem, 1)
                        vector.wait_ge(asem, a_exp0)
                        scale_rows(0, 0, 128).then_inc(vsem, 1)
                vector.wait_ge(psem, p_last(NCHUNK - 1))
                for j in range(2, 4):
                    ins = negmax_piece(1, j)
                ins.then_inc(vsem, 1)
                vector.wait_ge(asem, a_exp1a)
                scale_rows(1, 0, 64).then_inc(vsem, 1)
                vector.wait_ge(asem, a_exp1b)
                scale_rows(1, 64, 128).then_inc(vsem, 1)

    return nc


def _get_program():
    if "nc" not in _CACHE:
        _CACHE["nc"] = build_program()
    return _CACHE["nc"]


def _pmajor(a, nchunks):
    """(nchunks*128, X) -> (128, nchunks, X) partition-major layout."""
    x = a.reshape(nchunks, 128, a.shape[-1])
    return np.ascontiguousarray(x.transpose(1, 0, 2))


def _make_in_maps(inputs):
    import ml_dtypes

    query = np.asarray(inputs["query"], dtype=np.float32)
    decoder_states = np.asarray(inputs["decoder_states"], dtype=np.float32)
    Wq = np.asarray(inputs["Wq"], dtype=np.float32)
    Wh = np.asarray(inputs["Wh"], dtype=np.float32)
    wqh = _pmajor(np.vstack([Wq, Wh]), NWC)
    w2v = np.asarray(inputs["w2"], np.float32).reshape(H)
    w2oh = np.zeros((H, 32, 32), dtype=np.float32)
    w2oh[:, np.arange(32), np.arange(32)] = w2v[:, None]
    w2oh = w2oh.astype(ml_dtypes.bfloat16)
    bqh = np.ascontiguousarray(
        (np.asarray(inputs["bq"], np.float32)
         + np.asarray(inputs["bh"], np.float32)).reshape(H, 1)
    )
    qTf = query.transpose(1, 2, 0)  # (B, Q, L_q)
    qT = np.stack([_pmajor(qTf[b], NQC) for b in range(B)])
    in_maps = []
    for c in range(N_CORES):
        dslice = decoder_states[c * A_PER:(c + 1) * A_PER]
        # (D, B*A): column (b*A + a) holds decoder_states[a, b, :]
        dT = _pmajor(
            dslice.transpose(2, 1, 0).reshape(D_SIZE, NAB), NDC
        )
        in_maps.append({
            "qT": qT,
            "dT": dT,
            "wqh": wqh,
            "w2oh": w2oh,
            "bqh": bqh,
        })
    return in_maps


def kernel(query, decoder_states, query_mask, Wq, bq, Wh, bh, w2, b2):
    from concourse.bass_utils import run_bass_kernel_spmd

    mask = np.asarray(query_mask)
    nc = _get_program()
    in_maps = _make_in_maps({
        "query": query, "decoder_states": decoder_states,
        "Wq": Wq, "Wh": Wh, "w2": w2, "bq": bq, "bh": bh,
    })
    res = run_bass_kernel_spmd(nc, in_maps, list(range(N_CORES))).results
    perm = _row_perm()  # (A_PER, B) -> raw row
    out = np.empty((L_A, B, L_Q), dtype=np.float32)
    for c in range(N_CORES):
        out[c * A_PER:(c + 1) * A_PER] = res[c]["raw"][perm, :]

    if not mask.all():
        # exact post-exp masking + renormalization, host-side
        m = mask.T.astype(np.float32)  # (B, L_q)
        out = out * m[None, :, :]
        out = out / out.sum(axis=-1, keepdims=True)
    return out
